# revision 23
# baseline (speedup 1.0000x reference)
"""Tensor-parallel LlamaAttention (S=2048, HID=4096, NH=32, NKV=8) on 8 trn2 cores.

Sharding: core c owns q heads {c, c+8, c+16, c+24} (all four share kv head c)
and kv head c.  Projections + attention are fully local; avT (bf16,
[128d, 2048s] per head group) is AllGathered, then each core computes its 512
output columns of o_proj (column-parallel wo).

v2 design (from the ~518us baseline):
- all weight/x/agt DMAs are batched: the host pre-tiles x into
  [16, 128, 4096] (chunk-major, 8 hid-tiles per group) and wq/wo into
  [128, 16384] so each transfer is one large 2D descriptor.  Cuts the Sync
  engine's per-descriptor issue cost (~0.6us each) from ~250us to ~45us and
  removes the DMA-issue pacing stalls in phase 1.
- phase 2: the per-block rowsum matmul pass (~36us of PE) is gone.  exp
  blocks are accumulated on DVE (even blocks) and GPSIMD (odd blocks) into
  two SBUF accumulators; one ones-matmul per chunk reduces them across
  partitions into the broadcast denominator.  Scores are computed in PAIRED
  2-bank PSUM tiles ([128,1024]) and exp'd with a single ACT instruction per
  pair, halving the ACT per-instruction overhead (ACT would otherwise become
  the phase-2 pacer at ~687ns/block).  Diagonal blocks are computed full
  width; their dead columns are never read.
- PSUM: p1 chains 2 banks, score pairs 2x2 banks, pav/prs shared ring 2
  banks = 8.
- o_out is written bf16 (host converts to f32), agt gathers ride one DMA
  per (group, quarter), o_out one DMA per 4 seq tiles.
- collectives unchanged: early halves AllGather during phase-1 chunk 3,
  late halves trigger inline per (j,3) chunk; AG writes ride the gpsimd
  software DGE.

Self-contained: shapes/sharding hardcoded; host does transposes/casts.
"""

from contextlib import ExitStack

import numpy as np
import ml_dtypes

import concourse.bacc as bacc
import concourse.tile as tile
import concourse.mybir as mybir
from concourse.bass_utils import run_bass_kernel_spmd

S = 2048
HID = 4096
NH = 32
NKV = 8
HD = 128
HALF = 64
N_CORES = 8
NREP = NH // NKV  # 4 q heads per core
NHT = HID // 128  # 32 hidden tiles
NST = S // 128    # 16 seq tiles
NSC = S // 512    # 4 seq chunks
NG = 4            # x/wq DMA groups per chunk (8 hid tiles each)
BF16 = mybir.dt.bfloat16
F32 = mybir.dt.float32

_CACHE = {}


def build_nc():
    nc = bacc.Bacc("TRN2", target_bir_lowering=False, debug=False,
                   num_devices=N_CORES)

    xG = nc.dram_tensor("xG", [NSC * NG, 128, 8 * 512], BF16,
                        kind="ExternalInput").ap()
    wq = nc.dram_tensor("wqT", [128, NHT * 512], BF16, kind="ExternalInput").ap()
    wk = nc.dram_tensor("wkT", [128, NHT * 128], BF16, kind="ExternalInput").ap()
    wv = nc.dram_tensor("wvT", [128, NHT * 128], BF16, kind="ExternalInput").ap()
    wo = nc.dram_tensor("woT", [128, NHT * 512], BF16, kind="ExternalInput").ap()
    cosT = nc.dram_tensor("cosT", [HD, S], F32, kind="ExternalInput").ap()
    sinT = nc.dram_tensor("sinT", [HD, S], F32, kind="ExternalInput").ap()
    tri = nc.dram_tensor("triT", [128, 128], BF16, kind="ExternalInput").ap()

    o_out = nc.dram_tensor("o_out", [S, 512], BF16, kind="ExternalOutput").ap()

    # groups 0/1: one full AllGather each (trigger early in the tail block).
    # groups 2/3: split into an early half (q-chunks 0-1, norms done by the
    # (j,1) block, AllGathered during phase-1 chunk 3 while the CC core is
    # idle) and a late half (q-chunks 2-3) — phase 3's early quarters then
    # never wait on a collective, and the late pieces have ~100us of margin.
    agh_in = {(j, h): nc.dram_tensor(f"agh_in{j}_{h}", [HD, S // 2],
                                     BF16).ap()
              for j in range(NREP) for h in (0, 1)}
    agh_out = {(j, h): nc.dram_tensor(f"agh_out{j}_{h}",
                                      [N_CORES * HD, S // 2], BF16,
                                      addr_space="Shared").ap()
               for j in range(NREP) for h in (0, 1)}

    with tile.TileContext(nc) as tc:
        _body(nc, tc, xG, wq, wk, wv, wo, cosT, sinT, tri,
              o_out, agh_in, agh_out)
    nc.compile()
    return nc


def _body(nc, tc, xG, wq, wk, wv, wo, cosT, sinT, tri,
          o_out, agh_in, agh_out):
    with tc.tile_pool(name="consts", bufs=1) as cpool:
        tri_sb = cpool.tile([128, 128], BF16, tag="tri")
        ones_sb = cpool.tile([128, 128], BF16, tag="ones")
        nc.sync.dma_start(out=tri_sb[:], in_=tri[:])
        nc.vector.memset(ones_sb[:], 1.0)

        with ExitStack() as es:
            qkvpool = es.enter_context(tc.tile_pool(name="qkv", bufs=1))
            qT_sb = [qkvpool.tile([HD, S], BF16, tag=f"qT{j}", name=f"qT{j}")
                     for j in range(NREP)]
            kT_sb = qkvpool.tile([HD, S], BF16, tag="kT")
            v_sb = qkvpool.tile([128, S], BF16, tag="v")  # col blk kt = s tile

            ppool = es.enter_context(tc.tile_pool(name="probs", bufs=6))
            avcpool = es.enter_context(tc.tile_pool(name="avc", bufs=8))
            spool = es.enter_context(tc.tile_pool(name="small", bufs=2))
            accpool = es.enter_context(tc.tile_pool(name="acc", bufs=2))
            agq = {}
            es_p2 = es.enter_context(ExitStack())
            pspp = es_p2.enter_context(
                tc.tile_pool(name="pspp", bufs=2, space="PSUM"))
            # shared 4-bank [128,512] ring: phase-1 accumulation chains and
            # phase-2 pav/prs flow through it; 4 slots so a chunk's first
            # av-matmul never waits on the previous chunk's norm tail
            ps2 = es_p2.enter_context(
                tc.tile_pool(name="ps2", bufs=4, space="PSUM"))
            p2 = _Phase2(nc, tc, qT_sb, kT_sb, v_sb, tri_sb, ones_sb,
                         agh_in, agh_out, agq, None,
                         ppool, avcpool, spool, accpool, pspp, ps2)
            with (
                tc.tile_pool(name="rconsts", bufs=1) as rcpool,
                tc.tile_pool(name="wproj", bufs=1) as wpool,
                tc.tile_pool(name="xc", bufs=8) as xpool,
                tc.tile_pool(name="rope", bufs=2) as rpool,
            ):
                p1 = _Phase1(nc, tc, xG, wq, wk, wv, cosT, sinT,
                             qT_sb, kT_sb, v_sb,
                             rcpool, wpool, xpool, rpool, ps2)
                p1.issue_dmas()
                p1.chunk(0)
                p1.chunk(1)
                for j in range(NREP):
                    p2.chunk(j, 0)
                p1.chunk(2)
                for j in range(NREP):
                    p2.chunk(j, 1)
                p1.chunk(3)
            # phase-1 pools closed; open the phase-3 pools in their space
            wopool = es.enter_context(tc.tile_pool(name="wo", bufs=1))
            agpool = es.enter_context(tc.tile_pool(name="ag", bufs=3))
            p2.agpool = agpool
            # early-half AllGathers for groups 2/3: inputs were written during
            # the (j,0)/(j,1) blocks; the gpsimd queue reaches these right
            # after, so the CC core churns through them during phase-1 chunk 3
            # (also acts as the core-alignment barrier)
            for jj in range(NREP):
                nc.gpsimd.collective_compute(
                    "AllGather", mybir.AluOpType.bypass,
                    replica_groups=[list(range(N_CORES))],
                    ins=[agh_in[(jj, 0)][:]], outs=[agh_out[(jj, 0)][:]])
            # o_proj weights load during the remaining phase-2 chunks
            wo_sb = wopool.tile([128, NHT * 512], BF16, tag="wo")
            for g in range(NG):
                nc.sync.dma_start(out=wo_sb[:, g * 4096:(g + 1) * 4096],
                                  in_=wo[:, g * 4096:(g + 1) * 4096])
            # agt prefetch at points where the AG is already complete; the
            # early-AG'd group 2/3 quarters go LAST so group 0/1's transfers
            # (needed first in phase 3) aren't queued behind their 4MB
            prefetch = {(2, 2): [(0, 0), (0, 1)], (2, 3): [(1, 0), (1, 1)],
                        (3, 2): [(2, 0), (2, 1)], (3, 3): [(3, 0), (3, 1)]}
            for j in range(NREP):
                for C in (2, 3):
                    for (jj, qq) in prefetch.get((j, C), ()):
                        p2.issue_agt(jj, qq)
                    p2.chunk(j, C)
            es_p2.close()  # free pspp/ps2 banks for phase 3
            opool = es.enter_context(tc.tile_pool(name="oout", bufs=2))
            po1 = es.enter_context(
                tc.tile_pool(name="po1", bufs=4, space="PSUM"))
            po2 = es.enter_context(
                tc.tile_pool(name="po2", bufs=4, space="PSUM"))
            _phase3(nc, tc, wo_sb, o_out, agq, agpool,
                    po1, po2, opool, p2.issue_agt)


class _Phase1:
    def __init__(self, nc, tc, xG, wq, wk, wv, cosT, sinT,
                 qT_sb, kT_sb, v_sb, rcpool, wpool, xpool, rpool, psmm):
        self.nc = nc
        self.xG, self.wq, self.wk, self.wv = xG, wq, wk, wv
        self.cosT, self.sinT = cosT, sinT
        self.qT_sb, self.kT_sb, self.v_sb = qT_sb, kT_sb, v_sb
        self.xpool, self.rpool, self.psmm = xpool, rpool, psmm
        self.cos_sb = rcpool.tile([HD, S], F32, tag="cos")
        self.sin_sb = rcpool.tile([HD, S], F32, tag="sin")
        self.wq_sb = wpool.tile([128, NHT * 512], BF16, tag="wq")
        self.wk_sb = wpool.tile([128, NHT * 128], BF16, tag="wk")
        self.wv_sb = wpool.tile([128, NHT * 128], BF16, tag="wv")
        self.xgs = {}

    def _x_dma(self, cs, g, h0, nh):
        # one DMA for hid tiles [h0, h0+nh) of chunk cs; they live inside
        # the 8-tile group tile g (sub-ranges share it via distinct names)
        nc = self.nc
        key = (cs, g)
        if key not in self.xgs:
            self.xgs[key] = self.xpool.tile([128, 8 * 512], BF16, tag="xg",
                                            name=f"xg{cs}_{g}")
        t = self.xgs[key]
        s0 = (h0 % 8) * 512
        nc.sync.dma_start(out=t[:, s0:s0 + nh * 512],
                          in_=self.xG[cs * NG + g][:, s0:s0 + nh * 512])

    def xt(self, cs, h):
        """[128, 512] AP for hid tile h of chunk cs."""
        return self.xgs[(cs, h // 8)][:, (h % 8) * 512:(h % 8 + 1) * 512]

    def issue_dmas(self):
        nc = self.nc
        # chunk 0 feeds the DMA-paced k+v prefix: wk first, then fine-grained
        # x pieces so the interleaved k/v chains start within ~4us; cos/sin
        # before wq so rope(k) isn't the q-chain gate; wq streams during k/v
        def wk_piece(c0, c1):
            nc.sync.dma_start(out=self.wk_sb[:, c0 * 128:c1 * 128],
                              in_=self.wk[:, c0 * 128:c1 * 128])
        # tiny leading pieces: the hw DMA round-robins the in-flight window,
        # so the first k-chain deps complete after ~1MB instead of ~2.5MB
        wk_piece(0, 4)
        self._x_dma(0, 0, 0, 1)
        self._x_dma(0, 0, 1, 1)
        wk_piece(4, 8)
        self._x_dma(0, 0, 2, 1)
        self._x_dma(0, 0, 3, 1)
        wk_piece(8, 16)
        self._x_dma(0, 0, 4, 4)
        wk_piece(16, 32)
        self._x_dma(0, 1, 8, 8)
        nc.sync.dma_start(out=self.cos_sb[:], in_=self.cosT[:])
        nc.sync.dma_start(out=self.sin_sb[:], in_=self.sinT[:])
        nc.sync.dma_start(out=self.wv_sb[:], in_=self.wv[:])
        self._x_dma(0, 2, 16, 8)
        self._x_dma(0, 3, 24, 8)
        for g in range(NG):
            nc.sync.dma_start(out=self.wq_sb[:, g * 4096:(g + 1) * 4096],
                              in_=self.wq[:, g * 4096:(g + 1) * 4096])
        for cs in range(1, NSC):
            for g in range(NG):
                self._x_dma(cs, g, g * 8, 8)

    def chunk(self, cs):
        nc = self.nc
        sc = slice(cs * 512, (cs + 1) * 512)
        psmm, rpool = self.psmm, self.rpool
        cos_sb, sin_sb = self.cos_sb, self.sin_sb
        MM = dict(skip_group_check=True)

        def _rope(dst, pp):
            # cos rows [0:64] == rows [64:128], so one full-width multiply
            # covers both cos terms; sin products land in matching partition
            # halves so the combine ops see equal SB base partitions
            tc_ = rpool.tile([128, 512], F32, tag="t1")
            nc.vector.tensor_mul(tc_[:], pp[:, :], cos_sb[:, sc])
            ts = rpool.tile([128, 512], F32, tag="t2")
            nc.vector.tensor_mul(ts[0:HALF, :], pp[HALF:128, :],
                                 sin_sb[0:HALF, sc])
            nc.vector.tensor_mul(ts[HALF:128, :], pp[0:HALF, :],
                                 sin_sb[HALF:128, sc])
            nc.vector.tensor_sub(dst[0:HALF, sc], tc_[0:HALF, :],
                                 ts[0:HALF, :])
            nc.vector.tensor_add(dst[HALF:128, sc], tc_[HALF:128, :],
                                 ts[HALF:128, :])

        def q_chains():
            for j in range(NREP):
                pq = psmm.tile([128, 512], F32, tag="ps2")
                for h in range(NHT):
                    nc.tensor.matmul(
                        pq[:],
                        self.wq_sb[:, h * 512 + j * 128:
                                   h * 512 + (j + 1) * 128],
                        self.xt(cs, h),
                        start=(h == 0), stop=(h == NHT - 1), **MM)
                _rope(self.qT_sb[j], pq)

        if cs == 0:
            # DMA-paced prefix: k chain first (smallest weight dep), then v,
            # then q chains against fully-landed wq
            pk = psmm.tile([128, 512], F32, tag="ps2")
            for h in range(NHT):
                nc.tensor.matmul(pk[:], self.wk_sb[:, h * 128:(h + 1) * 128],
                                 self.xt(cs, h),
                                 start=(h == 0), stop=(h == NHT - 1), **MM)
            _rope(self.kT_sb, pk)
            pv = psmm.tile([128, 512], F32, tag="ps2")
            for tl in range(4):
                for h in range(NHT):
                    nc.tensor.matmul(
                        pv[:, tl * 128:(tl + 1) * 128],
                        self.xt(cs, h)[:, tl * 128:(tl + 1) * 128],
                        self.wv_sb[:, h * 128:(h + 1) * 128],
                        start=(h == 0), stop=(h == NHT - 1), **MM)
            nc.scalar.copy(self.v_sb[:, sc], pv[:])
            q_chains()
            return

        q_chains()

        pk = psmm.tile([128, 512], F32, tag="ps2")
        for h in range(NHT):
            nc.tensor.matmul(pk[:], self.wk_sb[:, h * 128:(h + 1) * 128],
                             self.xt(cs, h),
                             start=(h == 0), stop=(h == NHT - 1), **MM)
        _rope(self.kT_sb, pk)

        pv = psmm.tile([128, 512], F32, tag="ps2")
        for tl in range(4):
            for h in range(NHT):
                nc.tensor.matmul(
                    pv[:, tl * 128:(tl + 1) * 128],
                    self.xt(cs, h)[:, tl * 128:(tl + 1) * 128],
                    self.wv_sb[:, h * 128:(h + 1) * 128],
                    start=(h == 0), stop=(h == NHT - 1), **MM)
        nc.scalar.copy(self.v_sb[:, sc], pv[:])


class _Phase2:
    def __init__(self, nc, tc, qT_sb, kT_sb, v_sb, tri_sb, ones_sb,
                 agh_in, agh_out, agq, agpool,
                 ppool, avcpool, spool, accpool, pspp, ps2):
        self.nc = nc
        self.qT_sb, self.kT_sb, self.v_sb = qT_sb, kT_sb, v_sb
        self.tri_sb, self.ones_sb = tri_sb, ones_sb
        self.agh_in, self.agh_out = agh_in, agh_out
        self.agq, self.agpool = agq, agpool
        self.ppool, self.avcpool, self.spool = ppool, avcpool, spool
        self.accpool = accpool
        self.pspp, self.ps2 = pspp, ps2

    def issue_agt(self, j, qq):
        nc = self.nc
        src = self.agh_out[(j, qq // 2)]
        col0 = (qq % 2) * 512
        # one DMA for all 8 r-blocks: [8*128, 1024] -> [128, 8, 512]
        t = self.agpool.tile([128, N_CORES * 512], BF16, tag=f"ag{j}",
                             name=f"ag{j}_{qq}")
        src3 = src.rearrange("(r p) c -> p r c", p=128)
        nc.sync.dma_start(
            out=t.rearrange("p (r c) -> p r c", c=512),
            in_=src3[:, :, col0:col0 + 512])
        self.agq[(j, qq)] = t

    def chunk(self, j, C):
        nc = self.nc
        Exp = mybir.ActivationFunctionType.Exp
        qc0 = C * 512
        qc = slice(qc0, qc0 + 512)
        nkt = 4 * C + 4
        prs = self.ps2.tile([128, 512], F32, tag="ps2", name=f"prs{j}_{C}")
        pav = self.ps2.tile([128, 512], F32, tag="ps2", name=f"pav{j}_{C}")
        # bf16 [128,1024] accumulators, one per vector engine; a full
        # (off-diagonal) pair is accumulated with a single 1024-wide op.
        # Lane/phase partials sum at most 4 exps each before the exact
        # f32 ones-matmul reduce, so bf16 rounding is ~eps/sqrt(128).
        acc_d = self.accpool.tile([128, 1024], BF16, tag="accd",
                                  name=f"accd{j}_{C}")
        acc_g = self.accpool.tile([128, 1024], BF16, tag="accg",
                                  name=f"accg{j}_{C}")
        nc.vector.memset(acc_d[:], 0.0)
        nc.gpsimd.memset(acc_g[:], 0.0)
        pend = []

        def drain_one():
            kt2, off2, ap2 = pend.pop(0)
            nc.tensor.matmul(pav[:, off2:512],
                             self.v_sb[:, kt2 * 128:(kt2 + 1) * 128],
                             ap2[:, off2:512],
                             start=(kt2 == 0), stop=(kt2 == nkt - 1),
                             skip_group_check=True)

        for m in range(nkt // 2):
            kt0, kt1 = 2 * m, 2 * m + 1
            off0 = max(0, (kt0 - 4 * C) * 128)
            off1 = max(0, (kt1 - 4 * C) * 128)
            pp = self.pspp.tile([128, 1024], F32, tag="pp",
                                name=f"pp{j}_{C}_{m}")
            nc.tensor.matmul(pp[:, 0:512],
                             self.kT_sb[:, kt0 * 128:(kt0 + 1) * 128],
                             self.qT_sb[j][:, qc],
                             start=True, stop=True, skip_group_check=True)
            nc.tensor.matmul(pp[:, 512:1024],
                             self.kT_sb[:, kt1 * 128:(kt1 + 1) * 128],
                             self.qT_sb[j][:, qc],
                             start=True, stop=True, skip_group_check=True)
            pt = self.ppool.tile([128, 1024], BF16, tag="pt",
                                 name=f"pt{j}_{C}_{m}")
            nc.scalar.activation(pt[:], pp[:], Exp)
            eng = nc.vector if m % 2 == 0 else nc.gpsimd
            acc = acc_d if m % 2 == 0 else acc_g
            if kt1 >= 4 * C:  # diagonal pair: mask k>q, ranged adds
                eng.tensor_mul(pt[:, off0:off0 + 128],
                               pt[:, off0:off0 + 128], self.tri_sb[:])
                eng.tensor_add(acc[:, off0:512], acc[:, off0:512],
                               pt[:, off0:512])
                eng.tensor_mul(pt[:, 512 + off1:512 + off1 + 128],
                               pt[:, 512 + off1:512 + off1 + 128],
                               self.tri_sb[:])
                eng.tensor_add(acc[:, 512 + off1:1024],
                               acc[:, 512 + off1:1024],
                               pt[:, 512 + off1:1024])
            else:
                eng.tensor_add(acc[:], acc[:], pt[:])
            pend.append((kt0, off0, pt[:, 0:512]))
            pend.append((kt1, off1, pt[:, 512:1024]))
            while len(pend) > 2:
                drain_one()
        while pend:
            drain_one()
        # denominator: fold the engines' accumulators, then reduce both
        # 512-phases across partitions with two accumulating ones-matmuls
        acs = self.spool.tile([128, 1024], BF16, tag="acs", name=f"acs{j}_{C}")
        nc.gpsimd.tensor_add(acs[:], acc_d[:], acc_g[:])
        nc.tensor.matmul(prs[:], self.ones_sb[:], acs[:, 0:512],
                         start=True, stop=False, skip_group_check=True)
        nc.tensor.matmul(prs[:], self.ones_sb[:], acs[:, 512:1024],
                         start=False, stop=True, skip_group_check=True)
        # normalization entirely off the PE path (DVE + gpsimd), inline
        bsb = self.spool.tile([128, 512], F32, tag="bsb", name=f"bsb{j}_{C}")
        nc.vector.reciprocal_approx_fast(out=bsb[:], in_=prs[:])
        avc = self.avcpool.tile([128, 512], BF16, tag="avc",
                                name=f"avc{j}_{C}")
        nc.vector.tensor_mul(avc[:], pav[:], bsb[:])
        half = C // 2
        hc = slice((C % 2) * 512, (C % 2) * 512 + 512)
        nc.gpsimd.dma_start(out=self.agh_in[(j, half)][:, hc], in_=avc[:])
        if C == NSC - 1:
            # late half; the early half's AG is issued in _body
            nc.gpsimd.collective_compute(
                "AllGather", mybir.AluOpType.bypass,
                replica_groups=[list(range(N_CORES))],
                ins=[self.agh_in[(j, 1)][:]],
                outs=[self.agh_out[(j, 1)][:]])

    def finish(self):
        pass


def _phase3(nc, tc, wo_sb, o_out, agq, agpool, po1, po2, opool,
            issue_agt):
    # remaining loads: quarters 2-3 (groups 2/3 gated by the late half-AGs,
    # which have ~100us of margin before their first consumer)
    for jj in range(NREP):
        issue_agt(jj, 2)
    for jj in range(NREP):
        issue_agt(jj, 3)

    po = {}

    def open_q(g):
        pool = po1 if g % 2 == 0 else po2
        tag = "po1" if g % 2 == 0 else "po2"
        for st in range(4 * g, 4 * g + 4):
            po[st] = pool.tile([128, 512], F32, tag=tag, name=f"po{st}")

    def run(g, j):
        for st in range(4 * g, 4 * g + 4):
            qq = st // 4
            c = st % 4
            t = agq[(j, qq)]
            for r in range(N_CORES):
                i = j * N_CORES + r
                nc.tensor.matmul(po[st][:],
                                 t[:, r * 512 + c * 128:
                                   r * 512 + (c + 1) * 128],
                                 wo_sb[:, i * 512:(i + 1) * 512],
                                 start=(i == 0), stop=(i == NHT - 1))

    def close_q(g):
        osb = opool.tile([128, 4 * 512], BF16, tag="o", name=f"o{g}")
        for st in range(4 * g, 4 * g + 4):
            c = st % 4
            nc.scalar.copy(osb[:, c * 512:(c + 1) * 512], po[st][:])
        dst = o_out.rearrange("(q p) c -> p q c", p=128)
        nc.sync.dma_start(out=dst[:, 4 * g:4 * g + 4, :],
                          in_=osb.rearrange("p (q c) -> p q c", c=512))

    open_q(0)
    for j in range(NREP):
        run(0, j)
    close_q(0)
    open_q(1)
    for j in range(NREP):
        run(1, j)
    close_q(1)
    open_q(2)
    run(2, 0)
    open_q(3)
    run(3, 0)
    run(2, 1)
    run(3, 1)
    run(2, 2)
    run(3, 2)
    run(2, 3)
    close_q(2)
    # final quarter: fuse the last accumulation pass with per-st copies so
    # the tail is one copy + one small DMA
    osb = opool.tile([128, 4 * 512], BF16, tag="o", name="o3f")
    t3 = agq[(3, 3)]
    for st in range(12, 16):
        c = st % 4
        for r in range(N_CORES):
            i = 3 * N_CORES + r
            nc.tensor.matmul(po[st][:],
                             t3[:, r * 512 + c * 128:r * 512 + (c + 1) * 128],
                             wo_sb[:, i * 512:(i + 1) * 512],
                             start=(i == 0), stop=(i == NHT - 1))
        nc.scalar.copy(osb[:, c * 512:(c + 1) * 512], po[st][:])
    dst = o_out.rearrange("(q p) c -> p q c", p=128)
    nc.sync.dma_start(out=dst[:, 12:16, :],
                      in_=osb.rearrange("p (q c) -> p q c", c=512))


def prep_inputs(hidden_states, wq, wk, wv, wo, cos, sin, causal_mask=None):
    bf16 = ml_dtypes.bfloat16
    x = np.asarray(hidden_states, np.float32)[0]          # (S, HID)
    xT = np.ascontiguousarray(x.T).astype(bf16)           # (HID, S)
    # pre-tile x for batched DMA: [cs, g, 128, 8*512] with 8 hid tiles per
    # group laid side by side
    xg = np.ascontiguousarray(
        xT.reshape(NG, 8, 128, NSC, 512).transpose(3, 0, 2, 1, 4)
        .reshape(NSC * NG, 128, 8 * 512))
    wq_s = (np.asarray(wq, np.float32) / np.sqrt(HD)).astype(np.float32)
    cos2 = np.asarray(cos, np.float32)[0, 0]              # (S, 64)
    sin2 = np.asarray(sin, np.float32)[0, 0]
    cosT = np.ascontiguousarray(np.concatenate([cos2.T, cos2.T], 0))  # (128,S)
    sinT = np.ascontiguousarray(np.concatenate([sin2.T, sin2.T], 0))
    kl = np.arange(128)[:, None]
    ql = np.arange(128)[None, :]
    triT = (kl <= ql).astype(bf16)                        # allow k <= q

    def tile128(w2d, blk):
        # (HID, blk) -> (128, NHT*blk): column block h holds rows of hid
        # tile h
        return np.ascontiguousarray(
            w2d.reshape(NHT, 128, blk).transpose(1, 0, 2).reshape(128, -1))

    # wo reordered to match AllGather row order: row p = j*1024 + r*128 + d
    # corresponds to head (j*8+r), dim d  ->  wo column (j*8+r)*128 + d.
    j_ = np.arange(NREP)[:, None, None]
    r_ = np.arange(N_CORES)[None, :, None]
    d_ = np.arange(HD)[None, None, :]
    col_order = ((j_ * N_CORES + r_) * HD + d_).reshape(-1)
    woT_full = np.ascontiguousarray(
        np.asarray(wo, np.float32)[:, col_order].T).astype(bf16)  # (4096c,HID)

    in_maps = []
    for c in range(N_CORES):
        heads = [jj * N_CORES + c for jj in range(NREP)]
        wq_rows = np.concatenate([wq_s[h * HD:(h + 1) * HD, :] for h in heads],
                                 0)
        wqT_c = tile128(np.ascontiguousarray(wq_rows.T).astype(bf16), 512)
        wkT_c = tile128(np.ascontiguousarray(
            np.asarray(wk, np.float32)[c * HD:(c + 1) * HD, :].T)
            .astype(bf16), 128)
        wvT_c = tile128(np.ascontiguousarray(
            np.asarray(wv, np.float32)[c * HD:(c + 1) * HD, :].T)
            .astype(bf16), 128)
        woT_c = tile128(np.ascontiguousarray(
            woT_full[:, c * 512:(c + 1) * 512]), 512)
        in_maps.append(dict(xG=xg, wqT=wqT_c, wkT=wkT_c, wvT=wvT_c,
                            woT=woT_c, cosT=cosT, sinT=sinT, triT=triT))
    return in_maps


def postprocess(results):
    out = np.empty((S, HID), np.float32)
    for c in range(N_CORES):
        out[:, c * 512:(c + 1) * 512] = results[c]["o_out"].astype(np.float32)
    return out[None]


def get_nc():
    if "nc" not in _CACHE:
        _CACHE["nc"] = build_nc()
    return _CACHE["nc"]


def kernel(hidden_states, wq, wk, wv, wo, cos, sin, causal_mask=None):
    nc = get_nc()
    in_maps = prep_inputs(hidden_states, wq, wk, wv, wo, cos, sin, causal_mask)
    res = run_bass_kernel_spmd(nc, in_maps, core_ids=list(range(N_CORES)))
    return postprocess(res.results)


# revision 25
# speedup vs baseline: 1.0101x; 1.0101x over previous
"""Tensor-parallel LlamaAttention (S=2048, HID=4096, NH=32, NKV=8) on 8 trn2 cores.

Sharding: core c owns q heads {c, c+8, c+16, c+24} (all four share kv head c)
and kv head c.  Projections + attention are fully local; avT (bf16,
[128d, 2048s] per head group) is AllGathered, then each core computes its 512
output columns of o_proj (column-parallel wo).

v2 design (from the ~518us baseline):
- all weight/x/agt DMAs are batched: the host pre-tiles x into
  [16, 128, 4096] (chunk-major, 8 hid-tiles per group) and wq/wo into
  [128, 16384] so each transfer is one large 2D descriptor.  Cuts the Sync
  engine's per-descriptor issue cost (~0.6us each) from ~250us to ~45us and
  removes the DMA-issue pacing stalls in phase 1.
- phase 2: the per-block rowsum matmul pass (~36us of PE) is gone.  exp
  blocks are accumulated on DVE (even blocks) and GPSIMD (odd blocks) into
  two SBUF accumulators; one ones-matmul per chunk reduces them across
  partitions into the broadcast denominator.  Scores are computed in PAIRED
  2-bank PSUM tiles ([128,1024]) and exp'd with a single ACT instruction per
  pair, halving the ACT per-instruction overhead (ACT would otherwise become
  the phase-2 pacer at ~687ns/block).  Diagonal blocks are computed full
  width; their dead columns are never read.
- PSUM: p1 chains 2 banks, score pairs 2x2 banks, pav/prs shared ring 2
  banks = 8.
- o_out is written bf16 (host converts to f32), agt gathers ride one DMA
  per (group, quarter), o_out one DMA per 4 seq tiles.
- collectives unchanged: early halves AllGather during phase-1 chunk 3,
  late halves trigger inline per (j,3) chunk; AG writes ride the gpsimd
  software DGE.

Self-contained: shapes/sharding hardcoded; host does transposes/casts.
"""

from contextlib import ExitStack

import numpy as np
import ml_dtypes

import concourse.bacc as bacc
import concourse.tile as tile
import concourse.mybir as mybir
from concourse.bass_utils import run_bass_kernel_spmd

S = 2048
HID = 4096
NH = 32
NKV = 8
HD = 128
HALF = 64
N_CORES = 8
NREP = NH // NKV  # 4 q heads per core
NHT = HID // 128  # 32 hidden tiles
NST = S // 128    # 16 seq tiles
NSC = S // 512    # 4 seq chunks
NG = 4            # x/wq DMA groups per chunk (8 hid tiles each)
BF16 = mybir.dt.bfloat16
F32 = mybir.dt.float32

_CACHE = {}


def build_nc():
    nc = bacc.Bacc("TRN2", target_bir_lowering=False, debug=False,
                   num_devices=N_CORES)

    xG = nc.dram_tensor("xG", [NSC * NG, 128, 8 * 512], BF16,
                        kind="ExternalInput").ap()
    wq = nc.dram_tensor("wqT", [128, NHT * 512], BF16, kind="ExternalInput").ap()
    wk = nc.dram_tensor("wkT", [128, NHT * 128], BF16, kind="ExternalInput").ap()
    wv = nc.dram_tensor("wvT", [128, NHT * 128], BF16, kind="ExternalInput").ap()
    wo = nc.dram_tensor("woT", [128, NHT * 512], BF16, kind="ExternalInput").ap()
    cosT = nc.dram_tensor("cosT", [HD, S], F32, kind="ExternalInput").ap()
    sinT = nc.dram_tensor("sinT", [HD, S], F32, kind="ExternalInput").ap()
    tri = nc.dram_tensor("triT", [128, 128], BF16, kind="ExternalInput").ap()

    o_out = nc.dram_tensor("o_out", [S, 512], BF16, kind="ExternalOutput").ap()

    # groups 0/1: one full AllGather each (trigger early in the tail block).
    # groups 2/3: split into an early half (q-chunks 0-1, norms done by the
    # (j,1) block, AllGathered during phase-1 chunk 3 while the CC core is
    # idle) and a late half (q-chunks 2-3) — phase 3's early quarters then
    # never wait on a collective, and the late pieces have ~100us of margin.
    agh_in = {(j, h): nc.dram_tensor(f"agh_in{j}_{h}", [HD, S // 2],
                                     BF16).ap()
              for j in range(NREP) for h in (0, 1)}
    agh_out = {(j, h): nc.dram_tensor(f"agh_out{j}_{h}",
                                      [N_CORES * HD, S // 2], BF16,
                                      addr_space="Shared").ap()
               for j in range(NREP) for h in (0, 1)}

    with tile.TileContext(nc) as tc:
        _body(nc, tc, xG, wq, wk, wv, wo, cosT, sinT, tri,
              o_out, agh_in, agh_out)
    nc.compile()
    return nc


def _body(nc, tc, xG, wq, wk, wv, wo, cosT, sinT, tri,
          o_out, agh_in, agh_out):
    with tc.tile_pool(name="consts", bufs=1) as cpool:
        tri_sb = cpool.tile([128, 128], BF16, tag="tri")
        ones_sb = cpool.tile([128, 128], BF16, tag="ones")
        nc.sync.dma_start(out=tri_sb[:], in_=tri[:])
        nc.vector.memset(ones_sb[:], 1.0)

        with ExitStack() as es:
            qkvpool = es.enter_context(tc.tile_pool(name="qkv", bufs=1))
            qT_sb = [qkvpool.tile([HD, S], BF16, tag=f"qT{j}", name=f"qT{j}")
                     for j in range(NREP)]
            kT_sb = qkvpool.tile([HD, S], BF16, tag="kT")
            v_sb = qkvpool.tile([128, S], BF16, tag="v")  # col blk kt = s tile

            ppool = es.enter_context(tc.tile_pool(name="probs", bufs=6))
            avcpool = es.enter_context(tc.tile_pool(name="avc", bufs=8))
            spool = es.enter_context(tc.tile_pool(name="small", bufs=2))
            accpool = es.enter_context(tc.tile_pool(name="acc", bufs=2))
            agq = {}
            es_p2 = es.enter_context(ExitStack())
            pspp = es_p2.enter_context(
                tc.tile_pool(name="pspp", bufs=3, space="PSUM"))
            # shared 2-bank [128,512] ring: phase-1 accumulation chains and
            # phase-2 pav/prs alternate through it
            ps2 = es_p2.enter_context(
                tc.tile_pool(name="ps2", bufs=2, space="PSUM"))
            p2 = _Phase2(nc, tc, qT_sb, kT_sb, v_sb, tri_sb, ones_sb,
                         agh_in, agh_out, agq, None,
                         ppool, avcpool, spool, accpool, pspp, ps2)
            with (
                tc.tile_pool(name="rconsts", bufs=1) as rcpool,
                tc.tile_pool(name="wproj", bufs=1) as wpool,
                tc.tile_pool(name="xc", bufs=8) as xpool,
                tc.tile_pool(name="rope", bufs=2) as rpool,
            ):
                p1 = _Phase1(nc, tc, xG, wq, wk, wv, cosT, sinT,
                             qT_sb, kT_sb, v_sb,
                             rcpool, wpool, xpool, rpool, ps2)
                p1.issue_dmas()
                p1.chunk(0)
                p1.chunk(1)
                for j in range(NREP):
                    p2.chunk(j, 0)
                p1.chunk(2)
                for j in range(NREP):
                    p2.chunk(j, 1)
                p1.chunk(3)
            # phase-1 pools closed; open the phase-3 pools in their space
            wopool = es.enter_context(tc.tile_pool(name="wo", bufs=1))
            agpool = es.enter_context(tc.tile_pool(name="ag", bufs=3))
            p2.agpool = agpool
            # early-half AllGathers for groups 2/3: inputs were written during
            # the (j,0)/(j,1) blocks; the gpsimd queue reaches these right
            # after, so the CC core churns through them during phase-1 chunk 3
            # (also acts as the core-alignment barrier)
            for jj in range(NREP):
                nc.gpsimd.collective_compute(
                    "AllGather", mybir.AluOpType.bypass,
                    replica_groups=[list(range(N_CORES))],
                    ins=[agh_in[(jj, 0)][:]], outs=[agh_out[(jj, 0)][:]])
            # o_proj weights load during the remaining phase-2 chunks
            wo_sb = wopool.tile([128, NHT * 512], BF16, tag="wo")
            for g in range(NG):
                nc.sync.dma_start(out=wo_sb[:, g * 4096:(g + 1) * 4096],
                                  in_=wo[:, g * 4096:(g + 1) * 4096])
            # agt prefetch at points where the AG is already complete; the
            # early-AG'd group 2/3 quarters go LAST so group 0/1's transfers
            # (needed first in phase 3) aren't queued behind their 4MB
            prefetch = {(2, 2): [(0, 0), (0, 1)], (2, 3): [(1, 0), (1, 1)],
                        (3, 2): [(2, 0), (2, 1)], (3, 3): [(3, 0), (3, 1)]}
            for j in range(NREP):
                for C in (2, 3):
                    for (jj, qq) in prefetch.get((j, C), ()):
                        p2.issue_agt(jj, qq)
                    p2.chunk(j, C)
            es_p2.close()  # free pspp/ps2 banks for phase 3
            opool = es.enter_context(tc.tile_pool(name="oout", bufs=2))
            po1 = es.enter_context(
                tc.tile_pool(name="po1", bufs=4, space="PSUM"))
            po2 = es.enter_context(
                tc.tile_pool(name="po2", bufs=4, space="PSUM"))
            _phase3(nc, tc, wo_sb, o_out, agq, agpool,
                    po1, po2, opool, p2.issue_agt)


class _Phase1:
    def __init__(self, nc, tc, xG, wq, wk, wv, cosT, sinT,
                 qT_sb, kT_sb, v_sb, rcpool, wpool, xpool, rpool, psmm):
        self.nc = nc
        self.xG, self.wq, self.wk, self.wv = xG, wq, wk, wv
        self.cosT, self.sinT = cosT, sinT
        self.qT_sb, self.kT_sb, self.v_sb = qT_sb, kT_sb, v_sb
        self.xpool, self.rpool, self.psmm = xpool, rpool, psmm
        self.cos_sb = rcpool.tile([HD, S], F32, tag="cos")
        self.sin_sb = rcpool.tile([HD, S], F32, tag="sin")
        self.wq_sb = wpool.tile([128, NHT * 512], BF16, tag="wq")
        self.wk_sb = wpool.tile([128, NHT * 128], BF16, tag="wk")
        self.wv_sb = wpool.tile([128, NHT * 128], BF16, tag="wv")
        self.xgs = {}

    def _x_dma(self, cs, g, h0, nh):
        # one DMA for hid tiles [h0, h0+nh) of chunk cs; they live inside
        # the 8-tile group tile g (sub-ranges share it via distinct names)
        nc = self.nc
        key = (cs, g)
        if key not in self.xgs:
            self.xgs[key] = self.xpool.tile([128, 8 * 512], BF16, tag="xg",
                                            name=f"xg{cs}_{g}")
        t = self.xgs[key]
        s0 = (h0 % 8) * 512
        nc.sync.dma_start(out=t[:, s0:s0 + nh * 512],
                          in_=self.xG[cs * NG + g][:, s0:s0 + nh * 512])

    def xt(self, cs, h):
        """[128, 512] AP for hid tile h of chunk cs."""
        return self.xgs[(cs, h // 8)][:, (h % 8) * 512:(h % 8 + 1) * 512]

    def issue_dmas(self):
        nc = self.nc
        # chunk 0 feeds the DMA-paced k+v prefix: wk first, then fine-grained
        # x pieces so the interleaved k/v chains start within ~4us; cos/sin
        # before wq so rope(k) isn't the q-chain gate; wq streams during k/v
        def wk_piece(c0, c1):
            nc.sync.dma_start(out=self.wk_sb[:, c0 * 128:c1 * 128],
                              in_=self.wk[:, c0 * 128:c1 * 128])
        # tiny leading pieces: the hw DMA round-robins the in-flight window,
        # so the first k-chain deps complete after ~1MB instead of ~2.5MB
        wk_piece(0, 4)
        self._x_dma(0, 0, 0, 1)
        self._x_dma(0, 0, 1, 1)
        wk_piece(4, 8)
        self._x_dma(0, 0, 2, 1)
        self._x_dma(0, 0, 3, 1)
        wk_piece(8, 16)
        self._x_dma(0, 0, 4, 4)
        wk_piece(16, 32)
        self._x_dma(0, 1, 8, 8)
        nc.sync.dma_start(out=self.cos_sb[:], in_=self.cosT[:])
        nc.sync.dma_start(out=self.sin_sb[:], in_=self.sinT[:])
        nc.sync.dma_start(out=self.wv_sb[:], in_=self.wv[:])
        self._x_dma(0, 2, 16, 8)
        self._x_dma(0, 3, 24, 8)
        for g in range(NG):
            nc.sync.dma_start(out=self.wq_sb[:, g * 4096:(g + 1) * 4096],
                              in_=self.wq[:, g * 4096:(g + 1) * 4096])
        for cs in range(1, NSC):
            for g in range(NG):
                self._x_dma(cs, g, g * 8, 8)

    def chunk(self, cs):
        nc = self.nc
        sc = slice(cs * 512, (cs + 1) * 512)
        psmm, rpool = self.psmm, self.rpool
        cos_sb, sin_sb = self.cos_sb, self.sin_sb
        MM = dict(skip_group_check=True)

        def _rope(dst, pp):
            # cos rows [0:64] == rows [64:128], so one full-width multiply
            # covers both cos terms; sin products land in matching partition
            # halves so the combine ops see equal SB base partitions
            tc_ = rpool.tile([128, 512], F32, tag="t1")
            nc.vector.tensor_mul(tc_[:], pp[:, :], cos_sb[:, sc])
            ts = rpool.tile([128, 512], F32, tag="t2")
            nc.vector.tensor_mul(ts[0:HALF, :], pp[HALF:128, :],
                                 sin_sb[0:HALF, sc])
            nc.vector.tensor_mul(ts[HALF:128, :], pp[0:HALF, :],
                                 sin_sb[HALF:128, sc])
            nc.vector.tensor_sub(dst[0:HALF, sc], tc_[0:HALF, :],
                                 ts[0:HALF, :])
            nc.vector.tensor_add(dst[HALF:128, sc], tc_[HALF:128, :],
                                 ts[HALF:128, :])

        def q_chains():
            for j in range(NREP):
                pq = psmm.tile([128, 512], F32, tag="ps2")
                for h in range(NHT):
                    nc.tensor.matmul(
                        pq[:],
                        self.wq_sb[:, h * 512 + j * 128:
                                   h * 512 + (j + 1) * 128],
                        self.xt(cs, h),
                        start=(h == 0), stop=(h == NHT - 1), **MM)
                _rope(self.qT_sb[j], pq)

        if cs == 0:
            # DMA-paced prefix: k chain first (smallest weight dep), then v,
            # then q chains against fully-landed wq
            pk = psmm.tile([128, 512], F32, tag="ps2")
            for h in range(NHT):
                nc.tensor.matmul(pk[:], self.wk_sb[:, h * 128:(h + 1) * 128],
                                 self.xt(cs, h),
                                 start=(h == 0), stop=(h == NHT - 1), **MM)
            _rope(self.kT_sb, pk)
            pv = psmm.tile([128, 512], F32, tag="ps2")
            for tl in range(4):
                for h in range(NHT):
                    nc.tensor.matmul(
                        pv[:, tl * 128:(tl + 1) * 128],
                        self.xt(cs, h)[:, tl * 128:(tl + 1) * 128],
                        self.wv_sb[:, h * 128:(h + 1) * 128],
                        start=(h == 0), stop=(h == NHT - 1), **MM)
            nc.scalar.copy(self.v_sb[:, sc], pv[:])
            q_chains()
            return

        q_chains()

        pk = psmm.tile([128, 512], F32, tag="ps2")
        for h in range(NHT):
            nc.tensor.matmul(pk[:], self.wk_sb[:, h * 128:(h + 1) * 128],
                             self.xt(cs, h),
                             start=(h == 0), stop=(h == NHT - 1), **MM)
        _rope(self.kT_sb, pk)

        pv = psmm.tile([128, 512], F32, tag="ps2")
        for tl in range(4):
            for h in range(NHT):
                nc.tensor.matmul(
                    pv[:, tl * 128:(tl + 1) * 128],
                    self.xt(cs, h)[:, tl * 128:(tl + 1) * 128],
                    self.wv_sb[:, h * 128:(h + 1) * 128],
                    start=(h == 0), stop=(h == NHT - 1), **MM)
        nc.scalar.copy(self.v_sb[:, sc], pv[:])


class _Phase2:
    def __init__(self, nc, tc, qT_sb, kT_sb, v_sb, tri_sb, ones_sb,
                 agh_in, agh_out, agq, agpool,
                 ppool, avcpool, spool, accpool, pspp, ps2):
        self.nc = nc
        self.qT_sb, self.kT_sb, self.v_sb = qT_sb, kT_sb, v_sb
        self.tri_sb, self.ones_sb = tri_sb, ones_sb
        self.agh_in, self.agh_out = agh_in, agh_out
        self.agq, self.agpool = agq, agpool
        self.ppool, self.avcpool, self.spool = ppool, avcpool, spool
        self.accpool = accpool
        self.pspp, self.ps2 = pspp, ps2

    def issue_agt(self, j, qq):
        nc = self.nc
        src = self.agh_out[(j, qq // 2)]
        col0 = (qq % 2) * 512
        # one DMA for all 8 r-blocks: [8*128, 1024] -> [128, 8, 512]
        t = self.agpool.tile([128, N_CORES * 512], BF16, tag=f"ag{j}",
                             name=f"ag{j}_{qq}")
        src3 = src.rearrange("(r p) c -> p r c", p=128)
        nc.sync.dma_start(
            out=t.rearrange("p (r c) -> p r c", c=512),
            in_=src3[:, :, col0:col0 + 512])
        self.agq[(j, qq)] = t

    def chunk(self, j, C):
        nc = self.nc
        Exp = mybir.ActivationFunctionType.Exp
        qc0 = C * 512
        qc = slice(qc0, qc0 + 512)
        nkt = 4 * C + 4
        prs = self.ps2.tile([128, 512], F32, tag="ps2", name=f"prs{j}_{C}")
        pav = self.ps2.tile([128, 512], F32, tag="ps2", name=f"pav{j}_{C}")
        # bf16 [128,1024] accumulators, one per vector engine; a full
        # (off-diagonal) pair is accumulated with a single 1024-wide op.
        # Lane/phase partials sum at most 4 exps each before the exact
        # f32 ones-matmul reduce, so bf16 rounding is ~eps/sqrt(128).
        acc_d = self.accpool.tile([128, 1024], BF16, tag="accd",
                                  name=f"accd{j}_{C}")
        acc_g = self.accpool.tile([128, 1024], BF16, tag="accg",
                                  name=f"accg{j}_{C}")
        nc.vector.memset(acc_d[:], 0.0)
        nc.gpsimd.memset(acc_g[:], 0.0)
        pend = []

        def drain_one():
            kt2, off2, ap2 = pend.pop(0)
            nc.tensor.matmul(pav[:, off2:512],
                             self.v_sb[:, kt2 * 128:(kt2 + 1) * 128],
                             ap2[:, off2:512],
                             start=(kt2 == 0), stop=(kt2 == nkt - 1),
                             skip_group_check=True)

        for m in range(nkt // 2):
            kt0, kt1 = 2 * m, 2 * m + 1
            off0 = max(0, (kt0 - 4 * C) * 128)
            off1 = max(0, (kt1 - 4 * C) * 128)
            pp = self.pspp.tile([128, 1024], F32, tag="pp",
                                name=f"pp{j}_{C}_{m}")
            nc.tensor.matmul(pp[:, 0:512],
                             self.kT_sb[:, kt0 * 128:(kt0 + 1) * 128],
                             self.qT_sb[j][:, qc],
                             start=True, stop=True, skip_group_check=True)
            nc.tensor.matmul(pp[:, 512:1024],
                             self.kT_sb[:, kt1 * 128:(kt1 + 1) * 128],
                             self.qT_sb[j][:, qc],
                             start=True, stop=True, skip_group_check=True)
            pt = self.ppool.tile([128, 1024], BF16, tag="pt",
                                 name=f"pt{j}_{C}_{m}")
            nc.scalar.activation(pt[:], pp[:], Exp)
            eng = nc.vector if m % 2 == 0 else nc.gpsimd
            acc = acc_d if m % 2 == 0 else acc_g
            if kt1 >= 4 * C:  # diagonal pair: mask k>q, ranged adds
                eng.tensor_mul(pt[:, off0:off0 + 128],
                               pt[:, off0:off0 + 128], self.tri_sb[:])
                eng.tensor_add(acc[:, off0:512], acc[:, off0:512],
                               pt[:, off0:512])
                eng.tensor_mul(pt[:, 512 + off1:512 + off1 + 128],
                               pt[:, 512 + off1:512 + off1 + 128],
                               self.tri_sb[:])
                eng.tensor_add(acc[:, 512 + off1:1024],
                               acc[:, 512 + off1:1024],
                               pt[:, 512 + off1:1024])
            else:
                eng.tensor_add(acc[:], acc[:], pt[:])
            pend.append((kt0, off0, pt[:, 0:512]))
            pend.append((kt1, off1, pt[:, 512:1024]))
            while len(pend) > 2:
                drain_one()
        while pend:
            drain_one()
        # denominator: fold the engines' accumulators, then reduce both
        # 512-phases across partitions with two accumulating ones-matmuls
        acs = self.spool.tile([128, 1024], BF16, tag="acs", name=f"acs{j}_{C}")
        nc.vector.tensor_add(acs[:], acc_d[:], acc_g[:])
        nc.tensor.matmul(prs[:], self.ones_sb[:], acs[:, 0:512],
                         start=True, stop=False, skip_group_check=True)
        nc.tensor.matmul(prs[:], self.ones_sb[:], acs[:, 512:1024],
                         start=False, stop=True, skip_group_check=True)
        # normalization entirely off the PE path (DVE + gpsimd), inline
        bsb = self.spool.tile([128, 512], F32, tag="bsb", name=f"bsb{j}_{C}")
        nc.vector.reciprocal_approx_fast(out=bsb[:], in_=prs[:])
        avc = self.avcpool.tile([128, 512], BF16, tag="avc",
                                name=f"avc{j}_{C}")
        nc.vector.tensor_mul(avc[:], pav[:], bsb[:])
        half = C // 2
        hc = slice((C % 2) * 512, (C % 2) * 512 + 512)
        nc.gpsimd.dma_start(out=self.agh_in[(j, half)][:, hc], in_=avc[:])
        if C == NSC - 1:
            # late half; the early half's AG is issued in _body
            nc.gpsimd.collective_compute(
                "AllGather", mybir.AluOpType.bypass,
                replica_groups=[list(range(N_CORES))],
                ins=[self.agh_in[(j, 1)][:]],
                outs=[self.agh_out[(j, 1)][:]])

    def finish(self):
        pass


def _phase3(nc, tc, wo_sb, o_out, agq, agpool, po1, po2, opool,
            issue_agt):
    # remaining loads: quarters 2-3 (groups 2/3 gated by the late half-AGs,
    # which have ~100us of margin before their first consumer)
    for jj in range(NREP):
        issue_agt(jj, 2)
    for jj in range(NREP):
        issue_agt(jj, 3)

    po = {}

    def open_q(g):
        pool = po1 if g % 2 == 0 else po2
        tag = "po1" if g % 2 == 0 else "po2"
        for st in range(4 * g, 4 * g + 4):
            po[st] = pool.tile([128, 512], F32, tag=tag, name=f"po{st}")

    def run(g, j):
        for st in range(4 * g, 4 * g + 4):
            qq = st // 4
            c = st % 4
            t = agq[(j, qq)]
            for r in range(N_CORES):
                i = j * N_CORES + r
                nc.tensor.matmul(po[st][:],
                                 t[:, r * 512 + c * 128:
                                   r * 512 + (c + 1) * 128],
                                 wo_sb[:, i * 512:(i + 1) * 512],
                                 start=(i == 0), stop=(i == NHT - 1))

    def close_q(g):
        osb = opool.tile([128, 4 * 512], BF16, tag="o", name=f"o{g}")
        for st in range(4 * g, 4 * g + 4):
            c = st % 4
            nc.scalar.copy(osb[:, c * 512:(c + 1) * 512], po[st][:])
        dst = o_out.rearrange("(q p) c -> p q c", p=128)
        nc.sync.dma_start(out=dst[:, 4 * g:4 * g + 4, :],
                          in_=osb.rearrange("p (q c) -> p q c", c=512))

    open_q(0)
    for j in range(NREP):
        run(0, j)
    close_q(0)
    open_q(1)
    for j in range(NREP):
        run(1, j)
    close_q(1)
    open_q(2)
    run(2, 0)
    open_q(3)
    run(3, 0)
    run(2, 1)
    run(3, 1)
    run(2, 2)
    run(3, 2)
    run(2, 3)
    close_q(2)
    # final quarter: fuse the last accumulation pass with per-st copies so
    # the tail is one copy + one small DMA
    osb = opool.tile([128, 4 * 512], BF16, tag="o", name="o3f")
    t3 = agq[(3, 3)]
    for st in range(12, 16):
        c = st % 4
        for r in range(N_CORES):
            i = 3 * N_CORES + r
            nc.tensor.matmul(po[st][:],
                             t3[:, r * 512 + c * 128:r * 512 + (c + 1) * 128],
                             wo_sb[:, i * 512:(i + 1) * 512],
                             start=(i == 0), stop=(i == NHT - 1))
        nc.scalar.copy(osb[:, c * 512:(c + 1) * 512], po[st][:])
    dst = o_out.rearrange("(q p) c -> p q c", p=128)
    nc.sync.dma_start(out=dst[:, 12:16, :],
                      in_=osb.rearrange("p (q c) -> p q c", c=512))


def prep_inputs(hidden_states, wq, wk, wv, wo, cos, sin, causal_mask=None):
    bf16 = ml_dtypes.bfloat16
    x = np.asarray(hidden_states, np.float32)[0]          # (S, HID)
    xT = np.ascontiguousarray(x.T).astype(bf16)           # (HID, S)
    # pre-tile x for batched DMA: [cs, g, 128, 8*512] with 8 hid tiles per
    # group laid side by side
    xg = np.ascontiguousarray(
        xT.reshape(NG, 8, 128, NSC, 512).transpose(3, 0, 2, 1, 4)
        .reshape(NSC * NG, 128, 8 * 512))
    wq_s = (np.asarray(wq, np.float32) / np.sqrt(HD)).astype(np.float32)
    cos2 = np.asarray(cos, np.float32)[0, 0]              # (S, 64)
    sin2 = np.asarray(sin, np.float32)[0, 0]
    cosT = np.ascontiguousarray(np.concatenate([cos2.T, cos2.T], 0))  # (128,S)
    sinT = np.ascontiguousarray(np.concatenate([sin2.T, sin2.T], 0))
    kl = np.arange(128)[:, None]
    ql = np.arange(128)[None, :]
    triT = (kl <= ql).astype(bf16)                        # allow k <= q

    def tile128(w2d, blk):
        # (HID, blk) -> (128, NHT*blk): column block h holds rows of hid
        # tile h
        return np.ascontiguousarray(
            w2d.reshape(NHT, 128, blk).transpose(1, 0, 2).reshape(128, -1))

    # wo reordered to match AllGather row order: row p = j*1024 + r*128 + d
    # corresponds to head (j*8+r), dim d  ->  wo column (j*8+r)*128 + d.
    j_ = np.arange(NREP)[:, None, None]
    r_ = np.arange(N_CORES)[None, :, None]
    d_ = np.arange(HD)[None, None, :]
    col_order = ((j_ * N_CORES + r_) * HD + d_).reshape(-1)
    woT_full = np.ascontiguousarray(
        np.asarray(wo, np.float32)[:, col_order].T).astype(bf16)  # (4096c,HID)

    in_maps = []
    for c in range(N_CORES):
        heads = [jj * N_CORES + c for jj in range(NREP)]
        wq_rows = np.concatenate([wq_s[h * HD:(h + 1) * HD, :] for h in heads],
                                 0)
        wqT_c = tile128(np.ascontiguousarray(wq_rows.T).astype(bf16), 512)
        wkT_c = tile128(np.ascontiguousarray(
            np.asarray(wk, np.float32)[c * HD:(c + 1) * HD, :].T)
            .astype(bf16), 128)
        wvT_c = tile128(np.ascontiguousarray(
            np.asarray(wv, np.float32)[c * HD:(c + 1) * HD, :].T)
            .astype(bf16), 128)
        woT_c = tile128(np.ascontiguousarray(
            woT_full[:, c * 512:(c + 1) * 512]), 512)
        in_maps.append(dict(xG=xg, wqT=wqT_c, wkT=wkT_c, wvT=wvT_c,
                            woT=woT_c, cosT=cosT, sinT=sinT, triT=triT))
    return in_maps


def postprocess(results):
    out = np.empty((S, HID), np.float32)
    for c in range(N_CORES):
        out[:, c * 512:(c + 1) * 512] = results[c]["o_out"].astype(np.float32)
    return out[None]


def get_nc():
    if "nc" not in _CACHE:
        _CACHE["nc"] = build_nc()
    return _CACHE["nc"]


def kernel(hidden_states, wq, wk, wv, wo, cos, sin, causal_mask=None):
    nc = get_nc()
    in_maps = prep_inputs(hidden_states, wq, wk, wv, wo, cos, sin, causal_mask)
    res = run_bass_kernel_spmd(nc, in_maps, core_ids=list(range(N_CORES)))
    return postprocess(res.results)


# revision 26
# speedup vs baseline: 1.0362x; 1.0258x over previous
"""Tensor-parallel LlamaAttention (S=2048, HID=4096, NH=32, NKV=8) on 8 trn2 cores.

Sharding: core c owns q heads {c, c+8, c+16, c+24} (all four share kv head c)
and kv head c.  Projections + attention are fully local; avT (bf16,
[128d, 2048s] per head group) is AllGathered, then each core computes its 512
output columns of o_proj (column-parallel wo).

v2 design (from the ~518us baseline):
- all weight/x/agt DMAs are batched: the host pre-tiles x into
  [16, 128, 4096] (chunk-major, 8 hid-tiles per group) and wq/wo into
  [128, 16384] so each transfer is one large 2D descriptor.  Cuts the Sync
  engine's per-descriptor issue cost (~0.6us each) from ~250us to ~45us and
  removes the DMA-issue pacing stalls in phase 1.
- phase 2: the per-block rowsum matmul pass (~36us of PE) is gone.  exp
  blocks are accumulated on DVE (even blocks) and GPSIMD (odd blocks) into
  two SBUF accumulators; one ones-matmul per chunk reduces them across
  partitions into the broadcast denominator.  Scores are computed in PAIRED
  2-bank PSUM tiles ([128,1024]) and exp'd with a single ACT instruction per
  pair, halving the ACT per-instruction overhead (ACT would otherwise become
  the phase-2 pacer at ~687ns/block).  Diagonal blocks are computed full
  width; their dead columns are never read.
- PSUM: p1 chains 2 banks, score pairs 2x2 banks, pav/prs shared ring 2
  banks = 8.
- o_out is written bf16 (host converts to f32), agt gathers ride one DMA
  per (group, quarter), o_out one DMA per 4 seq tiles.
- collectives unchanged: early halves AllGather during phase-1 chunk 3,
  late halves trigger inline per (j,3) chunk; AG writes ride the gpsimd
  software DGE.

Self-contained: shapes/sharding hardcoded; host does transposes/casts.
"""

from contextlib import ExitStack

import numpy as np
import ml_dtypes

import concourse.bacc as bacc
import concourse.tile as tile
import concourse.mybir as mybir
from concourse.bass_utils import run_bass_kernel_spmd

S = 2048
HID = 4096
NH = 32
NKV = 8
HD = 128
HALF = 64
N_CORES = 8
NREP = NH // NKV  # 4 q heads per core
NHT = HID // 128  # 32 hidden tiles
NST = S // 128    # 16 seq tiles
NSC = S // 512    # 4 seq chunks
NG = 4            # x/wq DMA groups per chunk (8 hid tiles each)
BF16 = mybir.dt.bfloat16
F32 = mybir.dt.float32

_CACHE = {}


def build_nc():
    nc = bacc.Bacc("TRN2", target_bir_lowering=False, debug=False,
                   num_devices=N_CORES)

    xG = nc.dram_tensor("xG", [NSC * NG, 128, 8 * 512], BF16,
                        kind="ExternalInput").ap()
    wq = nc.dram_tensor("wqT", [128, NHT * 512], BF16, kind="ExternalInput").ap()
    wk = nc.dram_tensor("wkT", [128, NHT * 128], BF16, kind="ExternalInput").ap()
    wv = nc.dram_tensor("wvT", [128, NHT * 128], BF16, kind="ExternalInput").ap()
    wo = nc.dram_tensor("woT", [128, NHT * 512], BF16, kind="ExternalInput").ap()
    cosT = nc.dram_tensor("cosT", [HD, S], F32, kind="ExternalInput").ap()
    sinT = nc.dram_tensor("sinT", [HD, S], F32, kind="ExternalInput").ap()
    tri = nc.dram_tensor("triT", [128, 128], BF16, kind="ExternalInput").ap()

    o_out = nc.dram_tensor("o_out", [S, 512], BF16, kind="ExternalOutput").ap()

    # groups 0/1: one full AllGather each (trigger early in the tail block).
    # groups 2/3: split into an early half (q-chunks 0-1, norms done by the
    # (j,1) block, AllGathered during phase-1 chunk 3 while the CC core is
    # idle) and a late half (q-chunks 2-3) — phase 3's early quarters then
    # never wait on a collective, and the late pieces have ~100us of margin.
    agh_in = {(j, h): nc.dram_tensor(f"agh_in{j}_{h}", [HD, S // 2],
                                     BF16).ap()
              for j in range(NREP) for h in (0, 1)}
    agh_out = {(j, h): nc.dram_tensor(f"agh_out{j}_{h}",
                                      [N_CORES * HD, S // 2], BF16,
                                      addr_space="Shared").ap()
               for j in range(NREP) for h in (0, 1)}

    with tile.TileContext(nc) as tc:
        _body(nc, tc, xG, wq, wk, wv, wo, cosT, sinT, tri,
              o_out, agh_in, agh_out)
    nc.compile()
    return nc


def _body(nc, tc, xG, wq, wk, wv, wo, cosT, sinT, tri,
          o_out, agh_in, agh_out):
    with tc.tile_pool(name="consts", bufs=1) as cpool:
        tri_sb = cpool.tile([128, 128], BF16, tag="tri")
        ones_sb = cpool.tile([128, 128], BF16, tag="ones")
        nc.sync.dma_start(out=tri_sb[:], in_=tri[:])
        nc.vector.memset(ones_sb[:], 1.0)

        with ExitStack() as es:
            qkvpool = es.enter_context(tc.tile_pool(name="qkv", bufs=1))
            qT_sb = [qkvpool.tile([HD, S], BF16, tag=f"qT{j}", name=f"qT{j}")
                     for j in range(NREP)]
            kT_sb = qkvpool.tile([HD, S], BF16, tag="kT")
            v_sb = qkvpool.tile([128, S], BF16, tag="v")  # col blk kt = s tile

            ppool = es.enter_context(tc.tile_pool(name="probs", bufs=6))
            avcpool = es.enter_context(tc.tile_pool(name="avc", bufs=8))
            spool = es.enter_context(tc.tile_pool(name="small", bufs=2))
            accpool = es.enter_context(tc.tile_pool(name="acc", bufs=2))
            agq = {}
            es_p2 = es.enter_context(ExitStack())
            pspp = es_p2.enter_context(
                tc.tile_pool(name="pspp", bufs=3, space="PSUM"))
            # shared 2-bank [128,512] ring: phase-1 accumulation chains and
            # phase-2 pav/prs alternate through it
            ps2 = es_p2.enter_context(
                tc.tile_pool(name="ps2", bufs=2, space="PSUM"))
            p2 = _Phase2(nc, tc, qT_sb, kT_sb, v_sb, tri_sb, ones_sb,
                         agh_in, agh_out, agq, None,
                         ppool, avcpool, spool, accpool, pspp, ps2)
            with (
                tc.tile_pool(name="rconsts", bufs=1) as rcpool,
                tc.tile_pool(name="wproj", bufs=1) as wpool,
                tc.tile_pool(name="xc", bufs=8) as xpool,
                tc.tile_pool(name="rope", bufs=2) as rpool,
            ):
                p1 = _Phase1(nc, tc, xG, wq, wk, wv, cosT, sinT,
                             qT_sb, kT_sb, v_sb,
                             rcpool, wpool, xpool, rpool, ps2)
                p1.issue_dmas()
                p1.chunk(0)
                p1.chunk(1)
                for j in range(NREP):
                    p2.chunk(j, 0)
                p1.chunk(2)
                for j in range(NREP):
                    p2.chunk(j, 1)
                p1.chunk(3)
            # phase-1 pools closed; open the phase-3 pools in their space
            wopool = es.enter_context(tc.tile_pool(name="wo", bufs=1))
            agpool = es.enter_context(tc.tile_pool(name="ag", bufs=3))
            p2.agpool = agpool
            # early-half AllGathers for groups 2/3: inputs were written during
            # the (j,0)/(j,1) blocks; the gpsimd queue reaches these right
            # after, so the CC core churns through them during phase-1 chunk 3
            # (also acts as the core-alignment barrier)
            for jj in range(NREP):
                nc.gpsimd.collective_compute(
                    "AllGather", mybir.AluOpType.bypass,
                    replica_groups=[list(range(N_CORES))],
                    ins=[agh_in[(jj, 0)][:]], outs=[agh_out[(jj, 0)][:]])
            # o_proj weights load during the remaining phase-2 chunks
            wo_sb = wopool.tile([128, NHT * 512], BF16, tag="wo")
            for g in range(NG):
                nc.sync.dma_start(out=wo_sb[:, g * 4096:(g + 1) * 4096],
                                  in_=wo[:, g * 4096:(g + 1) * 4096])
            # agt prefetch at points where the AG is already complete; the
            # early-AG'd group 2/3 quarters go LAST so group 0/1's transfers
            # (needed first in phase 3) aren't queued behind their 4MB
            prefetch = {(2, 2): [(0, 0), (0, 1)], (2, 3): [(1, 0), (1, 1)],
                        (3, 2): [(2, 0), (2, 1)], (3, 3): [(3, 0), (3, 1)]}
            for j in range(NREP):
                for C in (2, 3):
                    for (jj, qq) in prefetch.get((j, C), ()):
                        p2.issue_agt(jj, qq)
                    p2.chunk(j, C)
            es_p2.close()  # free pspp/ps2 banks for phase 3
            opool = es.enter_context(tc.tile_pool(name="oout", bufs=2))
            po1 = es.enter_context(
                tc.tile_pool(name="po1", bufs=4, space="PSUM"))
            po2 = es.enter_context(
                tc.tile_pool(name="po2", bufs=4, space="PSUM"))
            _phase3(nc, tc, wo_sb, o_out, agq, agpool,
                    po1, po2, opool, p2.issue_agt)


class _Phase1:
    def __init__(self, nc, tc, xG, wq, wk, wv, cosT, sinT,
                 qT_sb, kT_sb, v_sb, rcpool, wpool, xpool, rpool, psmm):
        self.nc = nc
        self.xG, self.wq, self.wk, self.wv = xG, wq, wk, wv
        self.cosT, self.sinT = cosT, sinT
        self.qT_sb, self.kT_sb, self.v_sb = qT_sb, kT_sb, v_sb
        self.xpool, self.rpool, self.psmm = xpool, rpool, psmm
        self.cos_sb = rcpool.tile([HD, S], F32, tag="cos")
        self.sin_sb = rcpool.tile([HD, S], F32, tag="sin")
        self.wq_sb = wpool.tile([128, NHT * 512], BF16, tag="wq")
        self.wk_sb = wpool.tile([128, NHT * 128], BF16, tag="wk")
        self.wv_sb = wpool.tile([128, NHT * 128], BF16, tag="wv")
        self.xgs = {}

    def _x_dma(self, cs, g, h0, nh):
        # one DMA for hid tiles [h0, h0+nh) of chunk cs; they live inside
        # the 8-tile group tile g (sub-ranges share it via distinct names)
        nc = self.nc
        key = (cs, g)
        if key not in self.xgs:
            self.xgs[key] = self.xpool.tile([128, 8 * 512], BF16, tag="xg",
                                            name=f"xg{cs}_{g}")
        t = self.xgs[key]
        s0 = (h0 % 8) * 512
        nc.sync.dma_start(out=t[:, s0:s0 + nh * 512],
                          in_=self.xG[cs * NG + g][:, s0:s0 + nh * 512])

    def xt(self, cs, h):
        """[128, 512] AP for hid tile h of chunk cs."""
        return self.xgs[(cs, h // 8)][:, (h % 8) * 512:(h % 8 + 1) * 512]

    def issue_dmas(self):
        nc = self.nc
        # chunk 0 feeds the DMA-paced k+v prefix: wk first, then fine-grained
        # x pieces so the interleaved k/v chains start within ~4us; cos/sin
        # before wq so rope(k) isn't the q-chain gate; wq streams during k/v
        nc.sync.dma_start(out=self.wk_sb[:], in_=self.wk[:])
        for h0 in (0, 2, 4, 6):
            self._x_dma(0, 0, h0, 2)
        self._x_dma(0, 1, 8, 8)
        nc.sync.dma_start(out=self.wv_sb[:], in_=self.wv[:])
        self._x_dma(0, 2, 16, 8)
        self._x_dma(0, 3, 24, 8)
        for g in range(NG):
            nc.sync.dma_start(out=self.wq_sb[:, g * 4096:(g + 1) * 4096],
                              in_=self.wq[:, g * 4096:(g + 1) * 4096])
        nc.sync.dma_start(out=self.cos_sb[:], in_=self.cosT[:])
        nc.sync.dma_start(out=self.sin_sb[:], in_=self.sinT[:])
        for cs in range(1, NSC):
            for g in range(NG):
                self._x_dma(cs, g, g * 8, 8)

    def chunk(self, cs):
        nc = self.nc
        sc = slice(cs * 512, (cs + 1) * 512)
        psmm, rpool = self.psmm, self.rpool
        cos_sb, sin_sb = self.cos_sb, self.sin_sb
        MM = dict(skip_group_check=True)

        def _rope(dst, pp):
            # cos rows [0:64] == rows [64:128], so one full-width multiply
            # covers both cos terms; sin products land in matching partition
            # halves so the combine ops see equal SB base partitions
            tc_ = rpool.tile([128, 512], F32, tag="t1")
            nc.vector.tensor_mul(tc_[:], pp[:, :], cos_sb[:, sc])
            ts = rpool.tile([128, 512], F32, tag="t2")
            nc.vector.tensor_mul(ts[0:HALF, :], pp[HALF:128, :],
                                 sin_sb[0:HALF, sc])
            nc.vector.tensor_mul(ts[HALF:128, :], pp[0:HALF, :],
                                 sin_sb[HALF:128, sc])
            nc.vector.tensor_sub(dst[0:HALF, sc], tc_[0:HALF, :],
                                 ts[0:HALF, :])
            nc.vector.tensor_add(dst[HALF:128, sc], tc_[HALF:128, :],
                                 ts[HALF:128, :])

        def q_chains():
            for j in range(NREP):
                pq = psmm.tile([128, 512], F32, tag="ps2")
                for h in range(NHT):
                    nc.tensor.matmul(
                        pq[:],
                        self.wq_sb[:, h * 512 + j * 128:
                                   h * 512 + (j + 1) * 128],
                        self.xt(cs, h),
                        start=(h == 0), stop=(h == NHT - 1), **MM)
                _rope(self.qT_sb[j], pq)

        if cs == 0:
            # DMA-paced prefix: k chain first (smallest weight dep), then v,
            # then q chains against fully-landed wq
            pk = psmm.tile([128, 512], F32, tag="ps2")
            for h in range(NHT):
                nc.tensor.matmul(pk[:], self.wk_sb[:, h * 128:(h + 1) * 128],
                                 self.xt(cs, h),
                                 start=(h == 0), stop=(h == NHT - 1), **MM)
            _rope(self.kT_sb, pk)
            pv = psmm.tile([128, 512], F32, tag="ps2")
            for tl in range(4):
                for h in range(NHT):
                    nc.tensor.matmul(
                        pv[:, tl * 128:(tl + 1) * 128],
                        self.xt(cs, h)[:, tl * 128:(tl + 1) * 128],
                        self.wv_sb[:, h * 128:(h + 1) * 128],
                        start=(h == 0), stop=(h == NHT - 1), **MM)
            nc.scalar.copy(self.v_sb[:, sc], pv[:])
            q_chains()
            return

        q_chains()

        pk = psmm.tile([128, 512], F32, tag="ps2")
        for h in range(NHT):
            nc.tensor.matmul(pk[:], self.wk_sb[:, h * 128:(h + 1) * 128],
                             self.xt(cs, h),
                             start=(h == 0), stop=(h == NHT - 1), **MM)
        _rope(self.kT_sb, pk)

        pv = psmm.tile([128, 512], F32, tag="ps2")
        for tl in range(4):
            for h in range(NHT):
                nc.tensor.matmul(
                    pv[:, tl * 128:(tl + 1) * 128],
                    self.xt(cs, h)[:, tl * 128:(tl + 1) * 128],
                    self.wv_sb[:, h * 128:(h + 1) * 128],
                    start=(h == 0), stop=(h == NHT - 1), **MM)
        nc.scalar.copy(self.v_sb[:, sc], pv[:])


class _Phase2:
    def __init__(self, nc, tc, qT_sb, kT_sb, v_sb, tri_sb, ones_sb,
                 agh_in, agh_out, agq, agpool,
                 ppool, avcpool, spool, accpool, pspp, ps2):
        self.nc = nc
        self.qT_sb, self.kT_sb, self.v_sb = qT_sb, kT_sb, v_sb
        self.tri_sb, self.ones_sb = tri_sb, ones_sb
        self.agh_in, self.agh_out = agh_in, agh_out
        self.agq, self.agpool = agq, agpool
        self.ppool, self.avcpool, self.spool = ppool, avcpool, spool
        self.accpool = accpool
        self.pspp, self.ps2 = pspp, ps2

    def issue_agt(self, j, qq):
        nc = self.nc
        src = self.agh_out[(j, qq // 2)]
        col0 = (qq % 2) * 512
        # one DMA for all 8 r-blocks: [8*128, 1024] -> [128, 8, 512]
        t = self.agpool.tile([128, N_CORES * 512], BF16, tag=f"ag{j}",
                             name=f"ag{j}_{qq}")
        src3 = src.rearrange("(r p) c -> p r c", p=128)
        nc.sync.dma_start(
            out=t.rearrange("p (r c) -> p r c", c=512),
            in_=src3[:, :, col0:col0 + 512])
        self.agq[(j, qq)] = t

    def chunk(self, j, C):
        nc = self.nc
        Exp = mybir.ActivationFunctionType.Exp
        qc0 = C * 512
        qc = slice(qc0, qc0 + 512)
        nkt = 4 * C + 4
        prs = self.ps2.tile([128, 512], F32, tag="ps2", name=f"prs{j}_{C}")
        pav = self.ps2.tile([128, 512], F32, tag="ps2", name=f"pav{j}_{C}")
        # bf16 [128,1024] accumulators, one per vector engine; a full
        # (off-diagonal) pair is accumulated with a single 1024-wide op.
        # Lane/phase partials sum at most 4 exps each before the exact
        # f32 ones-matmul reduce, so bf16 rounding is ~eps/sqrt(128).
        acc_d = self.accpool.tile([128, 1024], BF16, tag="accd",
                                  name=f"accd{j}_{C}")
        acc_g = self.accpool.tile([128, 1024], BF16, tag="accg",
                                  name=f"accg{j}_{C}")
        nc.vector.memset(acc_d[:], 0.0)
        nc.gpsimd.memset(acc_g[:], 0.0)
        pend = []

        def drain_one():
            kt2, off2, ap2 = pend.pop(0)
            nc.tensor.matmul(pav[:, off2:512],
                             self.v_sb[:, kt2 * 128:(kt2 + 1) * 128],
                             ap2[:, off2:512],
                             start=(kt2 == 0), stop=(kt2 == nkt - 1),
                             skip_group_check=True)

        for m in range(nkt // 2):
            kt0, kt1 = 2 * m, 2 * m + 1
            off0 = max(0, (kt0 - 4 * C) * 128)
            off1 = max(0, (kt1 - 4 * C) * 128)
            pp = self.pspp.tile([128, 1024], F32, tag="pp",
                                name=f"pp{j}_{C}_{m}")
            nc.tensor.matmul(pp[:, 0:512],
                             self.kT_sb[:, kt0 * 128:(kt0 + 1) * 128],
                             self.qT_sb[j][:, qc],
                             start=True, stop=True, skip_group_check=True)
            nc.tensor.matmul(pp[:, 512:1024],
                             self.kT_sb[:, kt1 * 128:(kt1 + 1) * 128],
                             self.qT_sb[j][:, qc],
                             start=True, stop=True, skip_group_check=True)
            pt = self.ppool.tile([128, 1024], BF16, tag="pt",
                                 name=f"pt{j}_{C}_{m}")
            nc.scalar.activation(pt[:], pp[:], Exp)
            eng = nc.vector if m % 2 == 0 else nc.gpsimd
            acc = acc_d if m % 2 == 0 else acc_g
            if kt1 >= 4 * C:  # diagonal pair: mask k>q, ranged adds
                eng.tensor_mul(pt[:, off0:off0 + 128],
                               pt[:, off0:off0 + 128], self.tri_sb[:])
                eng.tensor_add(acc[:, off0:512], acc[:, off0:512],
                               pt[:, off0:512])
                eng.tensor_mul(pt[:, 512 + off1:512 + off1 + 128],
                               pt[:, 512 + off1:512 + off1 + 128],
                               self.tri_sb[:])
                eng.tensor_add(acc[:, 512 + off1:1024],
                               acc[:, 512 + off1:1024],
                               pt[:, 512 + off1:1024])
            else:
                eng.tensor_add(acc[:], acc[:], pt[:])
            pend.append((kt0, off0, pt[:, 0:512]))
            pend.append((kt1, off1, pt[:, 512:1024]))
            while len(pend) > 2:
                drain_one()
        while pend:
            drain_one()
        # denominator: fold the engines' accumulators, then reduce both
        # 512-phases across partitions with two accumulating ones-matmuls
        acs = self.spool.tile([128, 1024], BF16, tag="acs", name=f"acs{j}_{C}")
        nc.vector.tensor_add(acs[:], acc_d[:], acc_g[:])
        nc.tensor.matmul(prs[:], self.ones_sb[:], acs[:, 0:512],
                         start=True, stop=False, skip_group_check=True)
        nc.tensor.matmul(prs[:], self.ones_sb[:], acs[:, 512:1024],
                         start=False, stop=True, skip_group_check=True)
        # normalization entirely off the PE path (DVE + gpsimd), inline
        bsb = self.spool.tile([128, 512], F32, tag="bsb", name=f"bsb{j}_{C}")
        nc.vector.reciprocal_approx_fast(out=bsb[:], in_=prs[:])
        avc = self.avcpool.tile([128, 512], BF16, tag="avc",
                                name=f"avc{j}_{C}")
        nc.vector.tensor_mul(avc[:], pav[:], bsb[:])
        half = C // 2
        hc = slice((C % 2) * 512, (C % 2) * 512 + 512)
        nc.gpsimd.dma_start(out=self.agh_in[(j, half)][:, hc], in_=avc[:])
        if C == NSC - 1:
            # late half; the early half's AG is issued in _body
            nc.gpsimd.collective_compute(
                "AllGather", mybir.AluOpType.bypass,
                replica_groups=[list(range(N_CORES))],
                ins=[self.agh_in[(j, 1)][:]],
                outs=[self.agh_out[(j, 1)][:]])

    def finish(self):
        pass


def _phase3(nc, tc, wo_sb, o_out, agq, agpool, po1, po2, opool,
            issue_agt):
    # remaining loads: quarters 2-3 (groups 2/3 gated by the late half-AGs,
    # which have ~100us of margin before their first consumer)
    for jj in range(NREP):
        issue_agt(jj, 2)
    for jj in range(NREP):
        issue_agt(jj, 3)

    po = {}

    def open_q(g):
        pool = po1 if g % 2 == 0 else po2
        tag = "po1" if g % 2 == 0 else "po2"
        for st in range(4 * g, 4 * g + 4):
            po[st] = pool.tile([128, 512], F32, tag=tag, name=f"po{st}")

    def run(g, j):
        for st in range(4 * g, 4 * g + 4):
            qq = st // 4
            c = st % 4
            t = agq[(j, qq)]
            for r in range(N_CORES):
                i = j * N_CORES + r
                nc.tensor.matmul(po[st][:],
                                 t[:, r * 512 + c * 128:
                                   r * 512 + (c + 1) * 128],
                                 wo_sb[:, i * 512:(i + 1) * 512],
                                 start=(i == 0), stop=(i == NHT - 1))

    def close_q(g):
        osb = opool.tile([128, 4 * 512], BF16, tag="o", name=f"o{g}")
        for st in range(4 * g, 4 * g + 4):
            c = st % 4
            nc.scalar.copy(osb[:, c * 512:(c + 1) * 512], po[st][:])
        dst = o_out.rearrange("(q p) c -> p q c", p=128)
        nc.sync.dma_start(out=dst[:, 4 * g:4 * g + 4, :],
                          in_=osb.rearrange("p (q c) -> p q c", c=512))

    open_q(0)
    for j in range(NREP):
        run(0, j)
    close_q(0)
    open_q(1)
    for j in range(NREP):
        run(1, j)
    close_q(1)
    open_q(2)
    run(2, 0)
    open_q(3)
    run(3, 0)
    run(2, 1)
    run(3, 1)
    run(2, 2)
    run(3, 2)
    run(2, 3)
    close_q(2)
    # final quarter: fuse the last accumulation pass with per-st copies so
    # the tail is one copy + one small DMA
    osb = opool.tile([128, 4 * 512], BF16, tag="o", name="o3f")
    t3 = agq[(3, 3)]
    for st in range(12, 16):
        c = st % 4
        for r in range(N_CORES):
            i = 3 * N_CORES + r
            nc.tensor.matmul(po[st][:],
                             t3[:, r * 512 + c * 128:r * 512 + (c + 1) * 128],
                             wo_sb[:, i * 512:(i + 1) * 512],
                             start=(i == 0), stop=(i == NHT - 1))
        nc.scalar.copy(osb[:, c * 512:(c + 1) * 512], po[st][:])
    dst = o_out.rearrange("(q p) c -> p q c", p=128)
    nc.sync.dma_start(out=dst[:, 12:16, :],
                      in_=osb.rearrange("p (q c) -> p q c", c=512))


def prep_inputs(hidden_states, wq, wk, wv, wo, cos, sin, causal_mask=None):
    bf16 = ml_dtypes.bfloat16
    x = np.asarray(hidden_states, np.float32)[0]          # (S, HID)
    xT = np.ascontiguousarray(x.T).astype(bf16)           # (HID, S)
    # pre-tile x for batched DMA: [cs, g, 128, 8*512] with 8 hid tiles per
    # group laid side by side
    xg = np.ascontiguousarray(
        xT.reshape(NG, 8, 128, NSC, 512).transpose(3, 0, 2, 1, 4)
        .reshape(NSC * NG, 128, 8 * 512))
    wq_s = (np.asarray(wq, np.float32) / np.sqrt(HD)).astype(np.float32)
    cos2 = np.asarray(cos, np.float32)[0, 0]              # (S, 64)
    sin2 = np.asarray(sin, np.float32)[0, 0]
    cosT = np.ascontiguousarray(np.concatenate([cos2.T, cos2.T], 0))  # (128,S)
    sinT = np.ascontiguousarray(np.concatenate([sin2.T, sin2.T], 0))
    kl = np.arange(128)[:, None]
    ql = np.arange(128)[None, :]
    triT = (kl <= ql).astype(bf16)                        # allow k <= q

    def tile128(w2d, blk):
        # (HID, blk) -> (128, NHT*blk): column block h holds rows of hid
        # tile h
        return np.ascontiguousarray(
            w2d.reshape(NHT, 128, blk).transpose(1, 0, 2).reshape(128, -1))

    # wo reordered to match AllGather row order: row p = j*1024 + r*128 + d
    # corresponds to head (j*8+r), dim d  ->  wo column (j*8+r)*128 + d.
    j_ = np.arange(NREP)[:, None, None]
    r_ = np.arange(N_CORES)[None, :, None]
    d_ = np.arange(HD)[None, None, :]
    col_order = ((j_ * N_CORES + r_) * HD + d_).reshape(-1)
    woT_full = np.ascontiguousarray(
        np.asarray(wo, np.float32)[:, col_order].T).astype(bf16)  # (4096c,HID)

    in_maps = []
    for c in range(N_CORES):
        heads = [jj * N_CORES + c for jj in range(NREP)]
        wq_rows = np.concatenate([wq_s[h * HD:(h + 1) * HD, :] for h in heads],
                                 0)
        wqT_c = tile128(np.ascontiguousarray(wq_rows.T).astype(bf16), 512)
        wkT_c = tile128(np.ascontiguousarray(
            np.asarray(wk, np.float32)[c * HD:(c + 1) * HD, :].T)
            .astype(bf16), 128)
        wvT_c = tile128(np.ascontiguousarray(
            np.asarray(wv, np.float32)[c * HD:(c + 1) * HD, :].T)
            .astype(bf16), 128)
        woT_c = tile128(np.ascontiguousarray(
            woT_full[:, c * 512:(c + 1) * 512]), 512)
        in_maps.append(dict(xG=xg, wqT=wqT_c, wkT=wkT_c, wvT=wvT_c,
                            woT=woT_c, cosT=cosT, sinT=sinT, triT=triT))
    return in_maps


def postprocess(results):
    out = np.empty((S, HID), np.float32)
    for c in range(N_CORES):
        out[:, c * 512:(c + 1) * 512] = results[c]["o_out"].astype(np.float32)
    return out[None]


def get_nc():
    if "nc" not in _CACHE:
        _CACHE["nc"] = build_nc()
    return _CACHE["nc"]


def kernel(hidden_states, wq, wk, wv, wo, cos, sin, causal_mask=None):
    nc = get_nc()
    in_maps = prep_inputs(hidden_states, wq, wk, wv, wo, cos, sin, causal_mask)
    res = run_bass_kernel_spmd(nc, in_maps, core_ids=list(range(N_CORES)))
    return postprocess(res.results)
